# revision 3
# baseline (speedup 1.0000x reference)
"""Trainium2 kernel v2: y = relu((x - pb) @ W + b) with per-row top-K masking.

Data-parallel over rows across 8 cores. Per core, the matmul uses a
2-pass precision decomposition:
  main:  f16(x) @ f16(W)                       (1 fp16 PE pass)
  corr:  fp8(xlo*2^11) @ fp8(W*2^12)
       + fp8(x*2^5)    @ fp8(Wlo*2^18)         (1 fused DoubleRow fp8 pass)
  y = main + corr * 2^-23
giving ~1e-5 rms act error (verified in sim: end-to-end rel err ~4e-3).
Operands are pre-split/transposed once per call into DRAM caches; the
repeated body streams them. Top-K threshold per row via count binary
search on DVE/ACT (counts of y >= mid), masking pre-ReLU y directly.
"""
import sys
sys.path.insert(0, "/opt/trn_rl_repo")

import numpy as np
import concourse.bass as bass
import concourse.bacc as bacc
import concourse.mybir as mybir
from concourse.tile import TileContext
from concourse.masks import make_identity

F32 = mybir.dt.float32
F16 = mybir.dt.float16
BF16 = mybir.dt.bfloat16
FP8 = mybir.dt.float8e4

B_FULL, D_IN, N_FEAT, K_TOP = 16384, 4096, 4096, 128
N_CORES = 8

SXLO, SW8, SX8, SWLO = 2.0**11, 2.0**12, 2.0**5, 2.0**18
SCOMB = 2.0**-23  # 1/(SXLO*SW8) == 1/(SX8*SWLO)


def _chunks(n, c):
    out, i = [], 0
    while i < n:
        out.append((i, min(c, n - i)))
        i += c
    return out


def build_nc(B_core, D, F, K, n_iters=18, repeat=1, fb=512, ch=4, ss=4,
             debug_acts=False, skip_corr=False, skip_main=False):
    assert B_core % 128 == 0 and D % 128 == 0 and F % fb == 0
    n_r, n_d, n_fb = B_core // 128, D // 128, F // fb
    supers = [list(range(i, min(i + ss, n_r))) for i in range(0, n_r, ss)]

    nc = bacc.Bacc("TRN2", target_bir_lowering=False, debug=True)
    x = nc.dram_tensor("x", [B_core, D], F32, kind="ExternalInput")
    w = nc.dram_tensor("w", [D, F], F32, kind="ExternalInput")
    out = nc.dram_tensor("out", [B_core, F], F32, kind="ExternalOutput")
    acts_dbg = None
    if debug_acts:
        acts_dbg = nc.dram_tensor("acts_dbg", [B_core, F], F32,
                                  kind="ExternalOutput")

    with TileContext(nc) as tc:
        from contextlib import ExitStack
        ctx = ExitStack()
        dpool = ctx.enter_context(tc.tile_pool(name="dcache", bufs=1,
                                               space="DRAM"))
        # DRAM caches, partition-major so SBUF DMAs are order-preserving
        xhT_d = dpool.tile([128, n_d, B_core], F16)
        xp_d = dpool.tile([128, n_d, 2, B_core], FP8)
        wh_d = dpool.tile([128, n_d, F], F16)
        wp_d = dpool.tile([128, n_d, 2, F], FP8)

        # ---------------- prep phase (runs once) ----------------
        with tc.tile_pool(name="pconst", bufs=1) as pc_pool, \
             tc.tile_pool(name="pwsrc", bufs=2) as pw_pool, \
             tc.tile_pool(name="pwork", bufs=2) as pk_pool, \
             tc.tile_pool(name="pxsrc", bufs=2) as px_pool, \
             tc.tile_pool(name="pstage", bufs=2) as pst_pool, \
             tc.tile_pool(name="pps16", bufs=4, space="PSUM") as pt16_pool, \
             tc.tile_pool(name="pps8", bufs=4, space="PSUM") as pt8_pool:
            id16 = pc_pool.tile([128, 128], F16)
            make_identity(nc, id16[:])
            idbf = pc_pool.tile([128, 128], BF16)
            make_identity(nc, idbf[:])

            # W split: f16 hi + fp8 pair (W*SW8, Wlo*SWLO)
            FC = 2048
            for d in range(n_d):
                for f0, fn in _chunks(F, FC):
                    fsl = slice(f0, f0 + fn)
                    wsrc = pw_pool.tile([128, FC], F32, tag="wsrc")
                    nc.sync.dma_start(out=wsrc[:, :fn],
                                      in_=w[d * 128:(d + 1) * 128, fsl])
                    wh = pk_pool.tile([128, FC], F16, tag="wh")
                    nc.vector.tensor_copy(wh[:, :fn], wsrc[:, :fn])
                    nc.sync.dma_start(out=wh_d[:, d, fsl], in_=wh[:, :fn])
                    wp = pk_pool.tile([128, 2, FC], FP8, tag="wp")
                    nc.vector.tensor_scalar_mul(wp[:, 0, :fn], wsrc[:, :fn], SW8)
                    wlo = pk_pool.tile([128, FC], F32, tag="wlo")
                    nc.vector.tensor_tensor(out=wlo[:, :fn], in0=wsrc[:, :fn],
                                            in1=wh[:, :fn],
                                            op=mybir.AluOpType.subtract)
                    nc.vector.tensor_scalar_mul(wp[:, 1, :fn], wlo[:, :fn], SWLO)
                    nc.sync.dma_start(out=wp_d[:, d, :, fsl], in_=wp[:, :, :fn])

            # x split + transpose: f16 hi + fp8 pair (xlo*SXLO, x*SX8)
            for r in range(n_r):
                rsl = slice(r * 128, (r + 1) * 128)
                xr = px_pool.tile([128, D], F32, tag="xr")
                nc.sync.dma_start(out=xr[:], in_=x[rsl, :])
                xh = px_pool.tile([128, D], F16, tag="xh")
                nc.vector.tensor_copy(xh[:], xr[:])
                x8 = px_pool.tile([128, D], BF16, tag="x8")
                nc.vector.tensor_scalar_mul(x8[:], xr[:], SX8)
                xlo = px_pool.tile([128, D], F32, tag="xlo")
                nc.vector.tensor_tensor(out=xlo[:], in0=xr[:], in1=xh[:],
                                        op=mybir.AluOpType.subtract)
                xl8 = px_pool.tile([128, D], BF16, tag="xl8")
                nc.vector.tensor_scalar_mul(xl8[:], xlo[:], SXLO)

                sth = pst_pool.tile([128, n_d, 128], F16, tag="sth")
                stp = pst_pool.tile([128, n_d, 2, 128], FP8, tag="stp")
                for d in range(n_d):
                    dsl = slice(d * 128, (d + 1) * 128)
                    p16 = pt16_pool.tile([128, 128], F16, tag="p16")
                    nc.tensor.transpose(p16[:], xh[:, dsl], id16[:])
                    nc.scalar.copy(sth[:, d, :], p16[:])
                    p8a = pt8_pool.tile([128, 128], BF16, tag="p8")
                    nc.tensor.transpose(p8a[:], xl8[:, dsl], idbf[:])
                    nc.scalar.copy(stp[:, d, 0, :], p8a[:])
                    p8b = pt8_pool.tile([128, 128], BF16, tag="p8")
                    nc.tensor.transpose(p8b[:], x8[:, dsl], idbf[:])
                    nc.scalar.copy(stp[:, d, 1, :], p8b[:])
                nc.sync.dma_start(out=xhT_d[:, :, rsl], in_=sth[:])
                nc.sync.dma_start(out=xp_d[:, :, :, rsl], in_=stp[:])

        # ---------------- main phase (repeated) ----------------
        with tc.tile_pool(name="xh", bufs=ss + 2) as xh_pool, \
             tc.tile_pool(name="xp", bufs=ss + 2) as xp_pool, \
             tc.tile_pool(name="wh", bufs=2) as wh_pool, \
             tc.tile_pool(name="wp", bufs=2) as wp_pool, \
             tc.tile_pool(name="acts", bufs=ss + 1) as acts_pool, \
             tc.tile_pool(name="tmp", bufs=2) as tmp_pool, \
             tc.tile_pool(name="scr", bufs=1) as scr_pool, \
             tc.tile_pool(name="scra", bufs=1) as scra_pool, \
             tc.tile_pool(name="sm", bufs=24) as sm_pool, \
             tc.tile_pool(name="pmm", bufs=ss, space="PSUM") as pm_pool, \
             tc.tile_pool(name="pmc", bufs=ss, space="PSUM") as pcr_pool:
            for rep in range(repeat):
                for sup in supers:
                    ns = len(sup)
                    xh_t, xp_t, acts = {}, {}, {}
                    for r in sup:
                        rsl = slice(r * 128, (r + 1) * 128)
                        xh_t[r] = xh_pool.tile([128, n_d, 128], F16, tag="xh", name=f"xht{r}")
                        nc.sync.dma_start(out=xh_t[r][:], in_=xhT_d[:, :, rsl])
                        xp_t[r] = xp_pool.tile([128, n_d, 2, 128], FP8, tag="xp", name=f"xpt{r}")
                        nc.sync.dma_start(out=xp_t[r][:], in_=xp_d[:, :, :, rsl])
                        acts[r] = acts_pool.tile([128, F], F32, tag="acts", name=f"acts{r}")

                    for f in range(n_fb):
                        fsl = slice(f * fb, (f + 1) * fb)
                        pms = {r: pm_pool.tile([128, fb], F32, tag="pm", name=f"pm{r}")
                               for r in sup}
                        pcs = {r: pcr_pool.tile([128, fb], F32, tag="pc", name=f"pc{r}")
                               for r in sup}
                        for c0, cn in _chunks(n_d, ch):
                            csl = slice(c0, c0 + cn)
                            whc = wh_pool.tile([128, ch, fb], F16, tag="wh")
                            nc.sync.dma_start(out=whc[:, :cn, :],
                                              in_=wh_d[:, csl, fsl])
                            wpc = wp_pool.tile([128, ch, 2, fb], FP8, tag="wp")
                            nc.sync.dma_start(out=wpc[:, :cn, :, :],
                                              in_=wp_d[:, csl, :, fsl])
                            for dd in range(cn):
                                d = c0 + dd
                                first, last = d == 0, d == n_d - 1
                                for r in sup:
                                    if not skip_main:
                                        nc.tensor.matmul(
                                            pms[r][:], xh_t[r][:, d, :],
                                            whc[:, dd, :],
                                            start=first, stop=last)
                                    if not skip_corr:
                                        nc.tensor.matmul(
                                            pcs[r][:], xp_t[r][:, d, :, :],
                                            wpc[:, dd, :, :],
                                            start=first, stop=last,
                                            perf_mode=mybir.MatmulPerfMode.DoubleRow)
                        for r in sup:
                            if skip_corr:
                                nc.scalar.activation(
                                    acts[r][:, fsl], pms[r][:],
                                    mybir.ActivationFunctionType.Copy)
                                continue
                            if skip_main:
                                nc.scalar.activation(
                                    acts[r][:, fsl], pcs[r][:],
                                    mybir.ActivationFunctionType.Copy,
                                    scale=SCOMB)
                                continue
                            tmp = tmp_pool.tile([128, fb], F32, tag="tmp")
                            nc.scalar.activation(
                                tmp[:], pcs[r][:],
                                mybir.ActivationFunctionType.Copy, scale=SCOMB)
                            nc.vector.tensor_tensor(
                                out=acts[r][:, fsl], in0=pms[r][:], in1=tmp[:],
                                op=mybir.AluOpType.add)

                    if debug_acts:
                        for r in sup:
                            nc.sync.dma_start(
                                out=acts_dbg[r * 128:(r + 1) * 128, :],
                                in_=acts[r][:])

                    # ---- per-row K-th largest via count binary search ----
                    # invariant: count(y >= lo) >= K, count(y >= lo+wdt) < K
                    lo = sm_pool.tile([128, ns], F32, tag="sm")
                    nc.vector.memset(lo[:], 0.0)
                    wdt = sm_pool.tile([128, ns], F32, tag="sm")
                    for i, r in enumerate(sup):
                        nc.vector.reduce_max(out=wdt[:, i:i + 1],
                                             in_=acts[r][:],
                                             axis=mybir.AxisListType.X)
                    nc.vector.tensor_scalar(wdt[:], wdt[:], 1.0001, 1e-20,
                                            op0=mybir.AluOpType.mult,
                                            op1=mybir.AluOpType.add)
                    mid = sm_pool.tile([128, ns], F32, tag="sm")
                    nc.vector.tensor_scalar_mul(mid[:], wdt[:], 0.5)
                    cnt = sm_pool.tile([128, ns], F32, tag="sm")
                    tgw = sm_pool.tile([128, ns], F32, tag="sm")
                    for it in range(n_iters):
                        for i, r in enumerate(sup):
                            if ns < 4 or i < 3:
                                # DVE: exact count of y >= mid
                                scr = scr_pool.tile([128, F], FP8, tag="scr")
                                nc.vector.tensor_scalar(
                                    scr[:], acts[r][:], mid[:, i:i + 1], None,
                                    op0=mybir.AluOpType.is_ge,
                                    op1=mybir.AluOpType.add,
                                    accum_out=cnt[:, i:i + 1])
                            else:
                                # ACT: S = sum(sign(mid - y)); count=(F-S)/2
                                scr2 = scra_pool.tile([128, F], FP8, tag="scra")
                                nc.scalar.activation(
                                    scr2[:], acts[r][:],
                                    mybir.ActivationFunctionType.Sign,
                                    bias=mid[:, i:i + 1], scale=-1.0,
                                    accum_out=cnt[:, i:i + 1])
                                nc.vector.tensor_scalar(
                                    cnt[:, i:i + 1], cnt[:, i:i + 1],
                                    -0.5, float(F) / 2.0,
                                    op0=mybir.AluOpType.mult,
                                    op1=mybir.AluOpType.add)
                        # wdt *= .5; lo += (cnt >= K-.75)*wdt; mid = lo+.5*wdt
                        nc.vector.tensor_scalar_mul(wdt[:], wdt[:], 0.5)
                        nc.vector.scalar_tensor_tensor(
                            out=tgw[:], in0=cnt[:], scalar=float(K) - 0.75,
                            in1=wdt[:], op0=mybir.AluOpType.is_ge,
                            op1=mybir.AluOpType.mult)
                        nc.vector.tensor_tensor(out=lo[:], in0=lo[:],
                                                in1=tgw[:],
                                                op=mybir.AluOpType.add)
                        if it != n_iters - 1:
                            nc.vector.scalar_tensor_tensor(
                                out=mid[:], in0=wdt[:], scalar=0.5, in1=lo[:],
                                op0=mybir.AluOpType.mult,
                                op1=mybir.AluOpType.add)
                    # ---- mask: out = y * (y >= lo); lo > 0 so relu implied
                    for i, r in enumerate(sup):
                        nc.vector.scalar_tensor_tensor(
                            out=acts[r][:], in0=acts[r][:],
                            scalar=lo[:, i:i + 1], in1=acts[r][:],
                            op0=mybir.AluOpType.is_ge,
                            op1=mybir.AluOpType.mult)
                        nc.sync.dma_start(out=out[r * 128:(r + 1) * 128, :],
                                          in_=acts[r][:])
        ctx.close()

    nc.finalize()
    return nc


_NC_CACHE = {}


def _get_nc(key):
    if key not in _NC_CACHE:
        _NC_CACHE[key] = build_nc(*key)
    return _NC_CACHE[key]


def kernel(x, preencoder_bias, W_enc, b_enc):
    from concourse.bass_utils import run_bass_kernel_spmd
    x = np.asarray(x, dtype=np.float32)
    W = np.asarray(W_enc, dtype=np.float32)
    pb = np.asarray(preencoder_bias, dtype=np.float32)
    b = np.asarray(b_enc, dtype=np.float32)

    B, D = x.shape
    F = W.shape[1]
    assert (B, D, F) == (B_FULL, D_IN, N_FEAT)
    # fold biases: (x - pb) @ W + b == x @ W + (b - pb @ W)
    c = (b - pb @ W).astype(np.float32)
    if np.any(c != 0.0):
        # exact: augment contraction with a block where x_aug[:,D]=1,
        # W_aug[D,:]=c (rest zeros)
        pad = 128
        x_aug = np.zeros((B, D + pad), dtype=np.float32)
        x_aug[:, :D] = x
        x_aug[:, D] = 1.0
        W_aug = np.zeros((D + pad, F), dtype=np.float32)
        W_aug[:D] = W
        W_aug[D] = c
        x, W, D = x_aug, W_aug, D + pad

    B_core = B // N_CORES
    nc = _get_nc((B_core, D, F, K_TOP))
    in_maps = [{"x": np.ascontiguousarray(x[i * B_core:(i + 1) * B_core]),
                "w": W} for i in range(N_CORES)]
    res = run_bass_kernel_spmd(nc, in_maps, core_ids=list(range(N_CORES)))
    return np.concatenate([res.results[i]["out"] for i in range(N_CORES)],
                          axis=0)
